# revision 50
# baseline (speedup 1.0000x reference)
"""CrossAttentionSpatial Trainium2 kernel (fp8 DoubleRow + f32r rewrite).

Full-input contract: kernel(**inputs) takes the complete tensors as numpy
arrays and returns the full [8, 256, 64, 64] float32 output.

Sharding: data-parallel over batch B=8 across the 8 NeuronCores (one batch
element per core). Each core computes its element end-to-end; no collectives.

Per-core math (b fixed), with GroupNorm folded into the 1x1 convs
(alpha scales weight rows, beta folds into biases):
  q = fp8(Wq_bf' x_bf + qb)  [C, N]   bf16 matmuls, fp8 eviction
  k = fp8(Wk_r' c_r + kb)    [C, N]   f32r matmuls (1 cycle/row, no casts)
  vt = fp8(Wv_r' c_r)^T      [N, C]   (no bias; handled in epilogue)
  S[m, n] = k^T q            fp8 DoubleRow (2 chunk-planes, 0.5 cyc/row)
  E = exp(S/16); delta = E - 1 stored fp8 (DVE tensor_scalar, 2x_2p)
  num = Vsum[c] + (vt^T delta)[c, n]   Vsum exact from GN means (rank-1)
  den[n] = 4096 + (1^T delta)[n]       PE DoubleRow ones-reduction
  out = num * recip(den) + vb_tot[c]
The delta/Vsum split keeps fp8 quantization error on the small softmax
residual instead of the O(1) softmax weights; logits are O(1) so exp
needs no max-subtraction.

Schedule: the attention loop is paced by ACT (one 1024-wide exp per
m-chunk pair, ~1.04us each, 128 of them).  S pairs run SKEW=2 ahead of
the O/den stream in one flat software pipeline; each nj's epilogue is
deferred into the next nj's first steps; the q projection for nj+1 and
the second half of the v projection are interleaved into the loop on
PE/DVE slack.  The head overlaps input DMA (the serial ~343 GB/s DMA
device is the floor) with GN stats (DVE), f32r rounding (GPSIMD/ACT),
and a k-projection wave that accumulates per cond chunk.
"""

from contextlib import ExitStack

import numpy as np

import concourse.bass as bass
import concourse.tile as tile
from concourse import mybir
from concourse.bass_utils import run_bass_kernel_spmd

F32 = mybir.dt.float32
BF16 = mybir.dt.bfloat16
F32R = mybir.dt.float32r
FP8 = mybir.dt.float8e4

B = 8
C = 256          # x channels
E = 512          # cond channels
N = 4096         # H*W
GROUPS = 32
DX = C // GROUPS     # 8 channels per group (x)
DC = E // GROUPS     # 16 channels per group (cond)
EPS = 1e-5
SOFTMAX_SCALE = 1.0 / 16.0   # 1/sqrt(C)

P = 128
CKX = C // P         # 2 channel chunks of x
CKC = E // P         # 4 channel chunks of cond
NJ = N // 512        # 8 column chunks of 512
MI = N // P          # 32 m chunks of 128
NP = MI // 2         # 16 m-chunk pairs (DoubleRow)

AF = mybir.ActivationFunctionType
OP = mybir.AluOpType
PM = mybir.MatmulPerfMode

_CACHE = {}


def _split_multiwait_instructions(nc, max_waits=1):
    """This container's walrus build rejects >1 sync-wait per CTRL
    instruction. Split multi-wait instructions into single-wait carriers
    inserted just before, on the same engine."""
    ctr = 0
    for f in nc.m.functions:
        for bb in f.blocks:
            insts = bb.instructions
            new_list = []
            changed = False
            for ins in insts:
                si = ins.sync_info
                if si is not None and len(si.on_wait) > max_waits:
                    waits = list(si.on_wait)
                    head, tail = waits[:-max_waits], waits[-max_waits:]
                    for w in head:
                        is_drain = type(ins).__name__ == "InstDrain"
                        cls = mybir.InstDrain if is_drain else mybir.InstNoOp
                        c = cls(name=f"I-waitsplit-{ctr}", ins=[], outs=[])
                        ctr += 1
                        c.engine = ins.engine
                        c.sync_info = mybir.SyncInfo(on_wait=[w], on_update=[])
                        new_list.append(c)
                    ins.sync_info = mybir.SyncInfo(
                        on_wait=tail, on_update=list(si.on_update)
                    )
                    changed = True
                new_list.append(ins)
            if changed:
                bb.instructions = new_list
    return nc


def build_module(fixup=True):
    nc = bass.Bass(num_swdge_queues=4)

    x_d = nc.dram_tensor("x", [C, N], F32, kind="ExternalInput")
    cond_d = nc.dram_tensor("cond", [E, N], F32, kind="ExternalInput")
    wq_d = nc.dram_tensor("wq_t", [C, C], F32, kind="ExternalInput")   # q_w.T
    wk_d = nc.dram_tensor("wk_t", [E, C], F32, kind="ExternalInput")   # k_w.T
    wv_d = nc.dram_tensor("wv_t", [E, C], F32, kind="ExternalInput")   # v_w.T
    # gn weights/biases and q/k/v biases packed column-wise by the host:
    # cols 0:2 gnx_w, 2:4 gnx_b, 4:6 q_b, 6:8 k_b, 8:12 gnc_w, 12:16 gnc_b,
    # 16:18 v_b (chunked [2, P].T)
    params_d = nc.dram_tensor("params", [P, 18], F32, kind="ExternalInput")
    out_d = nc.dram_tensor("out", [C, N], F32, kind="ExternalOutput")

    # group-indicator constants for cross-partition group reductions
    gx = np.zeros((P, P // DX), np.float32)
    for c in range(P):
        gx[c, c // DX] = 1.0
    gc = np.zeros((P, P // DC), np.float32)
    for c in range(P):
        gc[c, c // DC] = 1.0
    t1 = np.zeros((P, 25), np.float32)
    t1[:, 0:16] = gx
    t1[:, 16:24] = gc
    t1[:, 24] = 1.0
    t2 = np.zeros((16, 384), np.float32)
    t2[:, 0:128] = gx.T
    t2[0:8, 128:256] = gc.T
    t2[0, 256:384] = 1.0
    t1_d = nc.inline_tensor(t1, "consts_col")
    t2_d = nc.inline_tensor(t2, "consts_row")

    with tile.TileContext(nc) as tc:
        with (
            tc.tile_pool(name="persist", bufs=1) as pp,
            tc.tile_pool(name="small", bufs=4) as smp,
        ):
            # ---- constants + params to SBUF ----
            t1_sb = pp.tile([P, 25], F32, tag="t1")
            t2_sb = pp.tile([16, 384], F32, tag="t2")
            pr_sb = pp.tile([P, 18], F32, tag="pr")
            nc.gpsimd.dma_start(out=t1_sb[:], in_=t1_d[:])
            nc.gpsimd.dma_start(out=t2_sb[:], in_=t2_d[:])
            nc.gpsimd.dma_start(out=pr_sb[:], in_=params_d[:])
            gx_sb = t1_sb[:, 0:16]
            gc_sb = t1_sb[:, 16:24]
            ones_col_f32 = t1_sb[:, 24:25]
            gxt_sb = t2_sb[:, 0:128]
            gct_sb = t2_sb[0:8, 128:256]
            ones_row_f32 = t2_sb[0:1, 256:384]
            gnxw = pr_sb[:, 0:2]
            gnxb = pr_sb[:, 2:4]
            qb_sb = pr_sb[:, 4:6]
            kb_sb = pr_sb[:, 6:8]
            gncw = pr_sb[:, 8:12]
            gncb = pr_sb[:, 12:16]
            vb_sb = pr_sb[:, 16:18]
            ones_row_r = pp.tile([1, P], F32R, tag="ones_row_r")
            ones8 = pp.tile([P, 2, 32], FP8, tag="ones8")
            eps_sb = pp.tile([P, 1], F32, tag="eps")
            with nc.allow_low_precision("f32r ones"):
                nc.vector.tensor_copy(ones_row_r[:], ones_row_f32[:])
            nc.vector.memset(ones8[:], 1.0)
            nc.vector.memset(eps_sb[:], EPS)

            NSUB = 8  # bn_stats free-dim limit is 512

            q_all = pp.tile([P, CKX, N], FP8, tag="q_all")
            k_all = pp.tile([P, CKX, N], FP8, tag="k_all")
            vt_all = pp.tile([P, MI, C], FP8, tag="vt_all")

            qb_f = pp.tile([P, CKX], F32, tag="qb_f")
            kb_f = pp.tile([P, CKX], F32, tag="kb_f")
            vsum = pp.tile([P, CKX], F32, tag="vsum")
            vbt = pp.tile([P, CKX], F32, tag="vbt")

            # x_r and wq_r persist: the q projection for nj+1 is pipelined
            # into the attention loop of nj.
            x_r = pp.tile([P, CKX, N], BF16, tag="x_r")
            wq_r = pp.tile([P, CKX, C], BF16, tag="wq_r")

            pj_stack = ExitStack()
            pj = pj_stack.enter_context(tc.tile_pool(name="projsb", bufs=1))
            head_stack = ExitStack()
            scp = head_stack.enter_context(tc.tile_pool(name="scratch", bufs=2))
            psp = head_stack.enter_context(
                tc.tile_pool(name="psum_small", bufs=1, space="PSUM"))
            phd = head_stack.enter_context(
                tc.tile_pool(name="psum_head", bufs=4, space="PSUM"))
            pwv = head_stack.enter_context(
                tc.tile_pool(name="psum_wave", bufs=3, space="PSUM"))
            if True:
                c_r = pj.tile([P, CKC, N], F32R, tag="c_r")
                wk_r = pj.tile([P, CKC, C], F32R, tag="wk_r")
                wv_r = pj.tile([P, CKC, C], F32R, tag="wv_r")
                alpha_x = pj.tile([P, CKX], F32, tag="alpha_x")
                beta_x = pj.tile([P, CKX], F32, tag="beta_x")
                alpha_c = pj.tile([P, CKC], F32, tag="alpha_c")
                beta_c = pj.tile([P, CKC], F32, tag="beta_c")
                rawcs = pj.tile([P, CKC], F32, tag="rawcs")  # 4096*raw mean
                rawcs2 = pj.tile([P, CKC], F32, tag="rawcs2")  # alpha-scaled

                # raw fp32 weights: dead after the bias folds in the head
                wq_f = pj.tile([P, CKX, C], F32, tag="wq_f")
                wk_f = pj.tile([P, CKC, C], F32, tag="wk_f")
                wv_f = pj.tile([P, CKC, C], F32, tag="wv_f")
                with (tc.tile_pool(name="raw", bufs=2) as rawp,
                      tc.tile_pool(name="rawx", bufs=2) as rawxp):

                    def affine_tail(me, k, g_sb, gt_sb, gpc, d_per_g,
                                    w_sb, b_sb, alpha, beta):
                        gs = psp.tile([gpc, 2], F32, tag="aff")
                        nc.tensor.matmul(gs[:], g_sb[:, :], me[:],
                                         start=True, stop=True)
                        mv = smp.tile([gpc, 2], F32, tag="mv")
                        nc.vector.tensor_scalar_mul(
                            out=mv[:], in0=gs[:], scalar1=1.0 / d_per_g
                        )
                        msq = smp.tile([gpc, 1], F32, tag="msq")
                        nc.vector.tensor_mul(msq[:], mv[:, 0:1], mv[:, 0:1])
                        var = smp.tile([gpc, 1], F32, tag="var")
                        nc.vector.tensor_sub(var[:], mv[:, 1:2], msq[:])
                        sd = smp.tile([gpc, 1], F32, tag="sd")
                        nc.scalar.activation(
                            out=sd[:], in_=var[:], func=AF.Sqrt,
                            bias=eps_sb[:gpc], scale=1.0,
                        )
                        mv2 = smp.tile([gpc, 2], F32, tag="mv2")
                        nc.vector.tensor_copy(mv2[:, 0:1], mv[:, 0:1])
                        nc.vector.reciprocal(mv2[:, 1:2], sd[:])
                        murs = psp.tile([P, 2], F32, tag="aff")
                        nc.tensor.matmul(
                            murs[:], gt_sb[:, :], mv2[:], start=True, stop=True
                        )
                        nc.vector.tensor_mul(
                            alpha[:, k : k + 1], murs[:, 1:2], w_sb[:, k : k + 1]
                        )
                        t1v = smp.tile([P, 1], F32, tag="t1v")
                        nc.vector.tensor_mul(t1v[:], murs[:, 0:1],
                                             alpha[:, k : k + 1])
                        nc.vector.tensor_sub(
                            beta[:, k : k + 1], b_sb[:, k : k + 1], t1v[:]
                        )

                    def process_chunk(dram, k, r_dst, g_sb, gt_sb, gpc,
                                      d_per_g, w_sb, b_sb, alpha, beta,
                                      mean_dst, queue_eng, round_act=False,
                                      defer_affine=False, use_rawx=False,
                                      defer_round=False):
                        # cond path: bn stats on DVE, f32r rounding on GPSIMD
                        H = N // 2
                        rp = rawxp if use_rawx else rawp
                        raw_a = rp.tile([P, H], F32, tag="rawa")
                        raw_b = rp.tile([P, H], F32, tag="rawb")
                        queue_eng.dma_start(
                            out=raw_a[:], in_=dram[k * P : (k + 1) * P, 0:H]
                        )
                        queue_eng.dma_start(
                            out=raw_b[:], in_=dram[k * P : (k + 1) * P, H:N]
                        )
                        bn = scp.tile([P, NSUB, 6], F32, tag="bn")
                        for s in range(NSUB):
                            src = raw_a if s < NSUB // 2 else raw_b
                            off = s % (NSUB // 2)
                            nc.vector.bn_stats(
                                out=bn[:, s, :],
                                in_=src[:, off * 512 : (off + 1) * 512],
                            )
                        if defer_round:
                            deferred_rounds.append((r_dst, k, raw_a, raw_b))
                        with nc.allow_low_precision("f32r inputs"):
                            if defer_round:
                                pass
                            elif round_act:
                                nc.scalar.activation(
                                    out=r_dst[:, k, 0:H], in_=raw_a[:],
                                    func=AF.Copy, scale=1.0)
                                nc.scalar.activation(
                                    out=r_dst[:, k, H:N], in_=raw_b[:],
                                    func=AF.Copy, scale=1.0)
                            else:
                                nc.gpsimd.tensor_copy(r_dst[:, k, 0:H],
                                                      raw_a[:])
                                nc.gpsimd.tensor_copy(r_dst[:, k, H:N],
                                                      raw_b[:])
                        mvp = scp.tile([P, 2], F32, tag="mvp")
                        nc.vector.bn_aggr(out=mvp[:], in_=bn[:])
                        if mean_dst is not None:
                            nc.vector.tensor_scalar(
                                out=mean_dst[:, k : k + 1], in0=mvp[:, 0:1],
                                scalar1=float(N), scalar2=None, op0=OP.mult,
                            )
                        # me = [mean, E[x^2]] per partition
                        me = scp.tile([P, 2], F32, tag="me",
                                      name=f"me_{dram.name}_{k}")
                        nc.vector.tensor_copy(me[:, 0:1], mvp[:, 0:1])
                        nc.vector.scalar_tensor_tensor(
                            out=me[:, 1:2], in0=mvp[:, 0:1], scalar=mvp[:, 0:1],
                            in1=mvp[:, 1:2], op0=OP.mult, op1=OP.add,
                        )
                        if defer_affine:
                            return me
                        affine_tail(me, k, g_sb, gt_sb, gpc, d_per_g,
                                    w_sb, b_sb, alpha, beta)

                    # cond chunks: DMAs alternate between the SP and ACT
                    # hwdge rings so two chunks stream in parallel.  Wave
                    # k-proj tiles accumulate per chunk to fill PE.
                    NWAVE = 3
                    wave_ps = [pwv.tile([P, 512], F32, tag="wave",
                                        name=f"wave_ps{w}")
                               for w in range(NWAVE)]
                    for k in range(CKC):
                        process_chunk(cond_d, k, c_r, gc_sb, gct_sb, P // DC,
                                      DC, gncw, gncb, alpha_c, beta_c, rawcs,
                                      nc.sync, round_act=(k == CKC - 1))
                        nc.sync.dma_start(
                            out=wk_f[:, k, :], in_=wk_d[k * P : (k + 1) * P, :])
                        with tc.high_priority():
                            with nc.allow_low_precision("f32r weights"):
                                nc.vector.tensor_scalar_mul(
                                    out=wk_r[:, k, :], in0=wk_f[:, k, :],
                                    scalar1=alpha_c[:, k : k + 1],
                                )
                        for w in range(NWAVE):
                            nc.tensor.matmul(
                                wave_ps[w][:],
                                wk_r[:, k, 0:P],
                                c_r[:, k, w * 512 : (w + 1) * 512],
                                start=(k == 0), stop=(k == CKC - 1),
                            )

                    for k in range(CKC):
                        nc.sync.dma_start(
                            out=wv_f[:, k, :], in_=wv_d[k * P : (k + 1) * P, :])
                    for k in range(CKX):
                        nc.sync.dma_start(
                            out=wq_f[:, k, :], in_=wq_d[k * P : (k + 1) * P, :])
                    # x chunks: DVE stats after cond, ACT rounding;
                    # the PE-dependent affine tail is deferred until after
                    # the k projection so it never blocks the PE queue
                    x_mes = []
                    deferred_rounds = []
                    for k in range(CKX):
                        x_mes.append(process_chunk(
                            x_d, k, x_r, gx_sb, gxt_sb,
                            P // DX, DX, gnxw, gnxb,
                            alpha_x, beta_x, None, nc.sync,
                            round_act=True, defer_affine=True,
                            use_rawx=True, defer_round=True))

                    with tc.high_priority():
                        with nc.allow_low_precision("f32r weights"):
                            for k in range(CKC):
                                nc.vector.tensor_scalar_mul(
                                    out=wv_r[:, k, :], in0=wv_f[:, k, :],
                                    scalar1=alpha_c[:, k : k + 1],
                                )

                    # k_b' fold and v-constants (cond-only)
                    for co in range(CKX):
                        bk = psp.tile([P, 1], F32, tag="aff")
                        for ci in range(CKC):
                            nc.tensor.matmul(
                                bk[:],
                                wk_f[:, ci, co * P : (co + 1) * P],
                                beta_c[:, ci : ci + 1],
                                start=(ci == 0), stop=(ci == CKC - 1),
                            )
                        nc.vector.tensor_add(
                            kb_f[:, co : co + 1], bk[:], kb_sb[:, co : co + 1]
                        )
                    # vb_tot[c] = v_b + Wv' beta_c ; Vsum[c] = Wv_r' rawcs
                    for co in range(CKX):
                        bv = psp.tile([P, 1], F32, tag="aff")
                        for ci in range(CKC):
                            nc.tensor.matmul(
                                bv[:],
                                wv_f[:, ci, co * P : (co + 1) * P],
                                beta_c[:, ci : ci + 1],
                                start=(ci == 0), stop=(ci == CKC - 1),
                            )
                        nc.vector.tensor_add(
                            vbt[:, co : co + 1], bv[:], vb_sb[:, co : co + 1]
                        )
                        if co == 0:
                            nc.vector.tensor_mul(rawcs2[:], rawcs[:],
                                                 alpha_c[:])
                        sv = psp.tile([P, 1], F32, tag="aff")
                        for ci in range(CKC):
                            nc.tensor.matmul(
                                sv[:],
                                wv_f[:, ci, co * P : (co + 1) * P],
                                rawcs2[:, ci : ci + 1],
                                start=(ci == 0), stop=(ci == CKC - 1),
                            )
                        nc.vector.tensor_copy(vsum[:, co : co + 1], sv[:])

                    # evict the wave tiles, then the remaining k-proj
                    for w in range(NWAVE):
                        nc.scalar.activation(
                            out=k_all[:, 0, w * 512 : (w + 1) * 512],
                            in_=wave_ps[w][:], func=AF.Identity,
                            bias=kb_f[:, 0:1], scale=1.0,
                        )
                    def krest_emit(co, nj, idx):
                        ps = phd.tile([P, 512], F32, tag="proj")
                        for ci in range(CKC):
                            nc.tensor.matmul(
                                ps[:],
                                wk_r[:, ci, co * P : (co + 1) * P],
                                c_r[:, ci, nj * 512 : (nj + 1) * 512],
                                start=(ci == 0), stop=(ci == CKC - 1),
                            )
                        if idx % 2 == 0:
                            nc.scalar.activation(
                                out=k_all[:, co, nj * 512 : (nj + 1) * 512],
                                in_=ps[:], func=AF.Identity,
                                bias=kb_f[:, co : co + 1], scale=1.0,
                            )
                        else:
                            nc.vector.tensor_scalar(
                                out=k_all[:, co, nj * 512 : (nj + 1) * 512],
                                in0=ps[:], scalar1=kb_f[:, co : co + 1],
                                scalar2=None, op0=OP.add,
                            )

                    def vproj_emit(pair, idx):
                        # two m-chunks share one psum tile and one eviction
                        ps = phd.tile([P, 2, C], F32, tag="proj")
                        for h in range(2):
                            mi = 2 * pair + h
                            for ci in range(CKC):
                                nc.tensor.matmul(
                                    ps[:, h, :],
                                    c_r[:, ci, mi * P : (mi + 1) * P],
                                    wv_r[:, ci, :],
                                    start=(ci == 0), stop=(ci == CKC - 1),
                                )
                        if idx % 2 == 0:
                            nc.vector.tensor_scalar(
                                out=vt_all[:, 2 * pair : 2 * pair + 2, :],
                                in0=ps[:],
                                scalar1=1.0, scalar2=None, op0=OP.mult,
                            )
                        else:
                            nc.scalar.activation(
                                out=vt_all[:, 2 * pair : 2 * pair + 2, :],
                                in_=ps[:], func=AF.Copy, scale=1.0,
                            )

                    krest = sorted(
                        ((co, nj) for co in range(CKX) for nj in range(NJ)
                         if not (co == 0 and nj < NWAVE)),
                        key=lambda t: (t[1] >= NJ // 2, t[1], t[0]))
                    idx = 0
                    vq = list(range(MI // 4))
                    for i, (co, nj) in enumerate(krest):
                        krest_emit(co, nj, idx); idx += 1
                        if i % 2 == 1 and vq:
                            vproj_emit(vq.pop(0), idx); idx += 1

                    # deferred x rounds: emitted after the k/v eviction
                    # chains so they never block the ACT queue while x still
                    # streams in
                    H2 = N // 2
                    for r_dst, kx, raw_a, raw_b in deferred_rounds:
                        with nc.allow_low_precision("f32r inputs"):
                            nc.scalar.activation(
                                out=r_dst[:, kx, 0:H2], in_=raw_a[:],
                                func=AF.Copy, scale=1.0)
                            nc.scalar.activation(
                                out=r_dst[:, kx, H2:N], in_=raw_b[:],
                                func=AF.Copy, scale=1.0)

                    # deferred x affine tails + q-weight prep
                    for k in range(CKX):
                        affine_tail(x_mes[k], k, gx_sb, gxt_sb, P // DX, DX,
                                    gnxw, gnxb, alpha_x, beta_x)
                    with nc.allow_low_precision("f32r weights"):
                        for k in range(CKX):
                            nc.vector.tensor_scalar_mul(
                                out=wq_r[:, k, :], in0=wq_f[:, k, :],
                                scalar1=alpha_x[:, k : k + 1],
                            )
                    for co in range(CKX):
                        bq = psp.tile([P, 1], F32, tag="aff")
                        for ci in range(CKX):
                            nc.tensor.matmul(
                                bq[:],
                                wq_f[:, ci, co * P : (co + 1) * P],
                                beta_x[:, ci : ci + 1],
                                start=(ci == 0), stop=(ci == CKX - 1),
                            )
                        nc.vector.tensor_add(
                            qb_f[:, co : co + 1], bq[:], qb_sb[:, co : co + 1]
                        )

                while vq:
                    vproj_emit(vq.pop(0), idx)
                    idx += 1
                # q projection for nj=0 only (the rest pipeline into the
                # attention loop); ACT eviction (ACT idle pre-attention)
                for co in range(CKX):
                    ps = phd.tile([P, 512], F32, tag="proj")
                    for ci in range(CKX):
                        nc.tensor.matmul(
                            ps[:],
                            wq_r[:, ci, co * P : (co + 1) * P],
                            x_r[:, ci, 0:512],
                            start=(ci == 0), stop=(ci == CKX - 1),
                        )
                    nc.scalar.activation(
                        out=q_all[:, co, 0:512],
                        in_=ps[:], func=AF.Identity,
                        bias=qb_f[:, co : co + 1], scale=1.0,
                    )

            head_stack.close()

            # ---- attention ----
            # Steady state is paced by ACT (one 1024-wide exp per m-pair).
            # The nj epilogue is deferred into the first steps of nj+1 so
            # the PE/DVE queues never stall behind the recip chain, and the
            # q projection for nj+1 runs mid-loop on PE/DVE slack.
            with (
                tc.tile_pool(name="psum_s", bufs=2, space="PSUM") as pss,
                tc.tile_pool(name="psum_o", bufs=1, space="PSUM") as pso,
                tc.tile_pool(name="psum_t", bufs=1, space="PSUM") as pst,
                tc.tile_pool(name="exps", bufs=4) as pex,
                tc.tile_pool(name="deltas", bufs=4) as pdl,
                tc.tile_pool(name="outs", bufs=1) as pout,
            ):
                SKEW = 2
                prev = None  # (nj, o0, o1, den_ps)

                def tail_a(state):
                    nj0, o0, o1, den_ps = state
                    den_sb = smp.tile([1, 512], F32, tag="den_sb",
                                      name=f"densb{nj0}")
                    nc.vector.tensor_scalar(
                        out=den_sb[:], in0=den_ps[0:1, :],
                        scalar1=float(N), scalar2=None, op0=OP.add,
                    )
                    recip = smp.tile([1, 512], F32R, tag="recip",
                                     name=f"recip{nj0}")
                    with nc.allow_low_precision("f32r reciprocal"):
                        nc.vector.reciprocal(recip[:], den_sb[:])
                    return recip

                def tail_b(state, recip):
                    nj0 = state[0]
                    bc_ps = pst.tile([P, 512], F32, tag="aux",
                                     name=f"bc{nj0}")
                    nc.tensor.matmul(
                        bc_ps[:], ones_row_r[:], recip[:],
                        start=True, stop=True,
                    )
                    bc_sb = pout.tile([P, 512], F32, tag="bc_sb",
                                      name=f"bcsb{nj0}")
                    nc.vector.tensor_copy(bc_sb[:], bc_ps[:])
                    return bc_sb

                def tail_c(state, bc_sb):
                    nj0, o0, o1, den_ps = state
                    for co in range(CKX):
                        o_sb = pout.tile([P, 512], F32, tag=f"osb{co}",
                                         name=f"osb{nj0}_{co}")
                        nc.vector.scalar_tensor_tensor(
                            out=o_sb[:], in0=(o0 if co == 0 else o1)[:],
                            scalar=vsum[:, co : co + 1], in1=bc_sb[:],
                            op0=OP.add, op1=OP.mult,
                        )
                        f_sb = pout.tile([P, 512], F32, tag=f"fsb{co}",
                                         name=f"fsb{nj0}_{co}")
                        nc.gpsimd.tensor_scalar(
                            out=f_sb[:], in0=o_sb[:],
                            scalar1=vbt[:, co : co + 1], scalar2=None,
                            op0=OP.add,
                        )
                        nc.sync.dma_start(
                            out=out_d[co * P : (co + 1) * P,
                                      nj0 * 512 : (nj0 + 1) * 512],
                            in_=f_sb[:],
                        )

                def qproj_emit(nj2, co):
                    ps = pst.tile([P, 512], F32, tag="aux",
                                  name=f"qproj{nj2}_{co}")
                    for ci in range(CKX):
                        nc.tensor.matmul(
                            ps[:],
                            wq_r[:, ci, co * P : (co + 1) * P],
                            x_r[:, ci, nj2 * 512 : (nj2 + 1) * 512],
                            start=(ci == 0), stop=(ci == CKX - 1),
                        )
                    nc.vector.tensor_scalar(
                        out=q_all[:, co, nj2 * 512 : (nj2 + 1) * 512],
                        in0=ps[:], scalar1=qb_f[:, co : co + 1],
                        scalar2=None, op0=OP.add,
                    )

                # One flat software pipeline over (nj, pair): the S/exp/
                # delta stream runs SKEW pairs ahead of the O/den stream,
                # crossing nj boundaries without draining.
                TOT = NJ * NP
                d_tiles = [None] * TOT
                o_tiles = {}
                recip = bc_sb = None
                for gs_ in range(TOT + SKEW):
                    if gs_ < TOT:
                        nj, pi = divmod(gs_, NP)
                        ncol = slice(nj * 512, (nj + 1) * 512)
                        s_pair = pss.tile([P, 2, 512], F32, tag="s")
                        for h in range(2):
                            mi = 2 * pi + h
                            nc.tensor.matmul(
                                s_pair[:, h, :],
                                k_all[:, :, mi * P : (mi + 1) * P],
                                q_all[:, :, ncol],
                                start=True, stop=True,
                                perf_mode=PM.DoubleRow,
                            )
                        e_pair = pex.tile([P, 2, 512], BF16, tag="e")
                        nc.scalar.activation(
                            out=e_pair[:], in_=s_pair[:], func=AF.Exp,
                            scale=SOFTMAX_SCALE,
                        )
                        d_pair = pdl.tile([P, 2, 512], FP8, tag="d")
                        nc.vector.tensor_scalar(
                            out=d_pair[:], in0=e_pair[:],
                            scalar1=1.0, scalar2=None, op0=OP.subtract,
                        )
                        d_tiles[gs_] = d_pair
                        # remaining v projection + q projection for
                        # nj+1 on PE/DVE slack
                        if nj == 0:
                            mi2 = 16 + pi
                            ps2 = pst.tile([P, C], F32, tag="aux",
                                           name=f"vproj{mi2}")
                            for ci in range(CKC):
                                nc.tensor.matmul(
                                    ps2[:],
                                    c_r[:, ci, mi2 * P : (mi2 + 1) * P],
                                    wv_r[:, ci, :],
                                    start=(ci == 0), stop=(ci == CKC - 1),
                                )
                            nc.vector.tensor_scalar(
                                out=vt_all[:, mi2, :], in0=ps2[:],
                                scalar1=1.0, scalar2=None, op0=OP.mult,
                            )
                        if nj < NJ - 1 and pi in (10, 12):
                            qproj_emit(nj + 1, 0 if pi == 10 else 1)
                    if gs_ >= SKEW:
                        go = gs_ - SKEW
                        nj, pi = divmod(go, NP)
                        if pi == 0:
                            # deferred epilogue of the previous nj
                            if prev is not None:
                                recip = tail_a(prev)
                                bc_sb = tail_b(prev, recip)
                                tail_c(prev, bc_sb)
                            o0 = pso.tile([P, 512], F32, tag="o0")
                            o1 = pso.tile([P, 512], F32, tag="o1")
                            den_ps = pst.tile([32, 512], F32, tag="den",
                                              name=f"den{nj}")
                            o_tiles[nj] = (nj, o0, o1, den_ps)
                        nj_, o0, o1, den_ps = o_tiles[nj]
                        d_pair = d_tiles[go]
                        st = pi == 0
                        sp = pi == NP - 1
                        nc.tensor.matmul(
                            o0[:], vt_all[:, 2 * pi : 2 * pi + 2, 0:P],
                            d_pair[:], start=st, stop=sp,
                            perf_mode=PM.DoubleRow,
                        )
                        nc.tensor.matmul(
                            o1[:], vt_all[:, 2 * pi : 2 * pi + 2, P:C],
                            d_pair[:], start=st, stop=sp,
                            perf_mode=PM.DoubleRow,
                        )
                        nc.tensor.matmul(
                            den_ps[:], ones8[:],
                            d_pair[:], start=st, stop=sp,
                            perf_mode=PM.DoubleRow,
                        )
                        d_tiles[go] = None
                        if sp:
                            prev = o_tiles.pop(nj)
                # final epilogue
                # final epilogue, split into column halves so the
                # serial den->recip->bc->scale chain pipelines
                nj0_, o0, o1, den_ps = prev
                for hh in range(2):
                    hs = slice(hh * 256, (hh + 1) * 256)
                    den_sb = smp.tile([1, 256], F32, tag="den_sb",
                                      name=f"densbf{hh}")
                    nc.vector.tensor_scalar(
                        out=den_sb[:], in0=den_ps[0:1, hs],
                        scalar1=float(N), scalar2=None, op0=OP.add,
                    )
                    recip = smp.tile([1, 256], F32R, tag="recip",
                                     name=f"recipf{hh}")
                    with nc.allow_low_precision("f32r reciprocal"):
                        nc.vector.reciprocal(recip[:], den_sb[:])
                    bc_ps = pst.tile([P, 256], F32, tag="aux",
                                     name=f"bcf{hh}")
                    nc.tensor.matmul(
                        bc_ps[:], ones_row_r[:], recip[:],
                        start=True, stop=True,
                    )
                    bc_sb = pout.tile([P, 256], F32, tag="bc_sb",
                                      name=f"bcsbf{hh}")
                    nc.vector.tensor_copy(bc_sb[:], bc_ps[:])
                    for co in range(CKX):
                        o_sb = pout.tile([P, 256], F32, tag=f"osb{co}",
                                         name=f"osbf{hh}_{co}")
                        nc.vector.scalar_tensor_tensor(
                            out=o_sb[:], in0=(o0 if co == 0 else o1)[:, hs],
                            scalar=vsum[:, co : co + 1], in1=bc_sb[:],
                            op0=OP.add, op1=OP.mult,
                        )
                        f_sb = pout.tile([P, 256], F32, tag=f"fsb{co}",
                                         name=f"fsbf{hh}_{co}")
                        nc.vector.tensor_scalar(
                            out=f_sb[:], in0=o_sb[:],
                            scalar1=vbt[:, co : co + 1], scalar2=None,
                            op0=OP.add,
                        )
                        nc.sync.dma_start(
                            out=out_d[co * P : (co + 1) * P,
                                      nj0_ * 512 + hh * 256 :
                                      nj0_ * 512 + (hh + 1) * 256],
                            in_=f_sb[:],
                        )
            pj_stack.close()

    nc.finalize()
    if fixup:
        _split_multiwait_instructions(nc)
    return nc


def pack_params(gn_x_w, gn_x_b, q_b, k_b, gn_c_w, gn_c_b, v_b):
    pr = np.zeros((P, 18), np.float32)
    pr[:, 0:2] = np.asarray(gn_x_w, np.float32).reshape(2, P).T
    pr[:, 2:4] = np.asarray(gn_x_b, np.float32).reshape(2, P).T
    pr[:, 4:6] = np.asarray(q_b, np.float32).reshape(2, P).T
    pr[:, 6:8] = np.asarray(k_b, np.float32).reshape(2, P).T
    pr[:, 8:12] = np.asarray(gn_c_w, np.float32).reshape(4, P).T
    pr[:, 12:16] = np.asarray(gn_c_b, np.float32).reshape(4, P).T
    pr[:, 16:18] = np.asarray(v_b, np.float32).reshape(2, P).T
    return pr


def _get_nc():
    if "nc" not in _CACHE:
        _CACHE["nc"] = build_module()
    return _CACHE["nc"]


def kernel(x, condA, gn_x_w, gn_x_b, gn_c_w, gn_c_b,
           q_w, q_b, k_w, k_b, v_w, v_b):
    x = np.asarray(x, np.float32)
    condA = np.asarray(condA, np.float32)
    wq_t = np.ascontiguousarray(np.asarray(q_w, np.float32).T)
    wk_t = np.ascontiguousarray(np.asarray(k_w, np.float32).T)
    wv_t = np.ascontiguousarray(np.asarray(v_w, np.float32).T)
    shared = {
        "wq_t": wq_t,
        "wk_t": wk_t,
        "wv_t": wv_t,
        "params": pack_params(gn_x_w, gn_x_b, q_b, k_b, gn_c_w, gn_c_b, v_b),
    }
    in_maps = []
    for b in range(B):
        m = dict(shared)
        m["x"] = np.ascontiguousarray(x[b].reshape(C, N))
        m["cond"] = np.ascontiguousarray(condA[b].reshape(E, N))
        in_maps.append(m)

    nc = _get_nc()
    res = run_bass_kernel_spmd(nc, in_maps, core_ids=list(range(B)))
    out = np.stack([res.results[b]["out"] for b in range(B)], axis=0)
    return out.reshape(B, C, 64, 64)


if __name__ == "__main__":
    rng = np.random.default_rng(0)
    ins = {
        "x": rng.standard_normal((B, C, 64, 64), dtype=np.float32),
        "condA": rng.standard_normal((B, E, 64, 64), dtype=np.float32),
        "gn_x_w": np.ones(C, np.float32),
        "gn_x_b": np.zeros(C, np.float32),
        "gn_c_w": np.ones(E, np.float32),
        "gn_c_b": np.zeros(E, np.float32),
        "q_w": (rng.standard_normal((C, C)) * 0.02).astype(np.float32),
        "q_b": np.zeros(C, np.float32),
        "k_w": (rng.standard_normal((C, E)) * 0.02).astype(np.float32),
        "k_b": np.zeros(C, np.float32),
        "v_w": (rng.standard_normal((C, E)) * 0.02).astype(np.float32),
        "v_b": np.zeros(C, np.float32),
    }
    o = kernel(**ins)
    print("out", o.shape, o.dtype, float(np.abs(o).max()))


# revision 53
# speedup vs baseline: 1.0017x; 1.0017x over previous
"""CrossAttentionSpatial Trainium2 kernel (fp8 DoubleRow + f32r rewrite).

Full-input contract: kernel(**inputs) takes the complete tensors as numpy
arrays and returns the full [8, 256, 64, 64] float32 output.

Sharding: data-parallel over batch B=8 across the 8 NeuronCores (one batch
element per core). Each core computes its element end-to-end; no collectives.

Per-core math (b fixed), with GroupNorm folded into the 1x1 convs
(alpha scales weight rows, beta folds into biases):
  q = fp8(Wq_bf' x_bf + qb)  [C, N]   bf16 matmuls, fp8 eviction
  k = fp8(Wk_r' c_r + kb)    [C, N]   f32r matmuls (1 cycle/row, no casts)
  vt = fp8(Wv_r' c_r)^T      [N, C]   (no bias; handled in epilogue)
  S[m, n] = k^T q            fp8 DoubleRow (2 chunk-planes, 0.5 cyc/row)
  E = exp(S/16); delta = E - 1 stored fp8 (DVE tensor_scalar, 2x_2p)
  num = Vsum[c] + (vt^T delta)[c, n]   Vsum exact from GN means (rank-1)
  den[n] = 4096 + (1^T delta)[n]       PE DoubleRow ones-reduction
  out = num * recip(den) + vb_tot[c]
The delta/Vsum split keeps fp8 quantization error on the small softmax
residual instead of the O(1) softmax weights; logits are O(1) so exp
needs no max-subtraction.

Schedule: the attention loop is paced by ACT (one 1024-wide exp per
m-chunk pair, ~1.04us each, 128 of them).  S pairs run SKEW=2 ahead of
the O/den stream in one flat software pipeline; each nj's epilogue is
deferred into the next nj's first steps; the q projection for nj+1 and
the second half of the v projection are interleaved into the loop on
PE/DVE slack.  The head overlaps input DMA (the serial ~343 GB/s DMA
device is the floor) with GN stats (DVE), f32r rounding (GPSIMD/ACT),
and a k-projection wave that accumulates per cond chunk.
"""

from contextlib import ExitStack

import numpy as np

import concourse.bass as bass
import concourse.tile as tile
from concourse import mybir
from concourse.bass_utils import run_bass_kernel_spmd

F32 = mybir.dt.float32
BF16 = mybir.dt.bfloat16
F32R = mybir.dt.float32r
FP8 = mybir.dt.float8e4

B = 8
C = 256          # x channels
E = 512          # cond channels
N = 4096         # H*W
GROUPS = 32
DX = C // GROUPS     # 8 channels per group (x)
DC = E // GROUPS     # 16 channels per group (cond)
EPS = 1e-5
SOFTMAX_SCALE = 1.0 / 16.0   # 1/sqrt(C)

P = 128
CKX = C // P         # 2 channel chunks of x
CKC = E // P         # 4 channel chunks of cond
NJ = N // 512        # 8 column chunks of 512
MI = N // P          # 32 m chunks of 128
NP = MI // 2         # 16 m-chunk pairs (DoubleRow)

AF = mybir.ActivationFunctionType
OP = mybir.AluOpType
PM = mybir.MatmulPerfMode

_CACHE = {}


def _split_multiwait_instructions(nc, max_waits=1):
    """This container's walrus build rejects >1 sync-wait per CTRL
    instruction. Split multi-wait instructions into single-wait carriers
    inserted just before, on the same engine."""
    ctr = 0
    for f in nc.m.functions:
        for bb in f.blocks:
            insts = bb.instructions
            new_list = []
            changed = False
            for ins in insts:
                si = ins.sync_info
                if si is not None and len(si.on_wait) > max_waits:
                    waits = list(si.on_wait)
                    head, tail = waits[:-max_waits], waits[-max_waits:]
                    for w in head:
                        is_drain = type(ins).__name__ == "InstDrain"
                        cls = mybir.InstDrain if is_drain else mybir.InstNoOp
                        c = cls(name=f"I-waitsplit-{ctr}", ins=[], outs=[])
                        ctr += 1
                        c.engine = ins.engine
                        c.sync_info = mybir.SyncInfo(on_wait=[w], on_update=[])
                        new_list.append(c)
                    ins.sync_info = mybir.SyncInfo(
                        on_wait=tail, on_update=list(si.on_update)
                    )
                    changed = True
                new_list.append(ins)
            if changed:
                bb.instructions = new_list
    return nc


def build_module(fixup=True):
    nc = bass.Bass(num_swdge_queues=4)

    x_d = nc.dram_tensor("x", [C, N], F32, kind="ExternalInput")
    cond_d = nc.dram_tensor("cond", [E, N], F32, kind="ExternalInput")
    wq_d = nc.dram_tensor("wq_t", [C, C], F32, kind="ExternalInput")   # q_w.T
    wk_d = nc.dram_tensor("wk_t", [E, C], F32, kind="ExternalInput")   # k_w.T
    wv_d = nc.dram_tensor("wv_t", [E, C], F32, kind="ExternalInput")   # v_w.T
    # gn weights/biases and q/k/v biases packed column-wise by the host:
    # cols 0:2 gnx_w, 2:4 gnx_b, 4:6 q_b, 6:8 k_b, 8:12 gnc_w, 12:16 gnc_b,
    # 16:18 v_b (chunked [2, P].T)
    params_d = nc.dram_tensor("params", [P, 18], F32, kind="ExternalInput")
    out_d = nc.dram_tensor("out", [C, N], F32, kind="ExternalOutput")

    # group-indicator constants for cross-partition group reductions
    gx = np.zeros((P, P // DX), np.float32)
    for c in range(P):
        gx[c, c // DX] = 1.0
    gc = np.zeros((P, P // DC), np.float32)
    for c in range(P):
        gc[c, c // DC] = 1.0
    t1 = np.zeros((P, 25), np.float32)
    t1[:, 0:16] = gx
    t1[:, 16:24] = gc
    t1[:, 24] = 1.0
    t2 = np.zeros((16, 384), np.float32)
    t2[:, 0:128] = gx.T
    t2[0:8, 128:256] = gc.T
    t2[0, 256:384] = 1.0
    t1_d = nc.inline_tensor(t1, "consts_col")
    t2_d = nc.inline_tensor(t2, "consts_row")

    with tile.TileContext(nc) as tc:
        with (
            tc.tile_pool(name="persist", bufs=1) as pp,
            tc.tile_pool(name="small", bufs=4) as smp,
        ):
            # ---- constants + params to SBUF ----
            t1_sb = pp.tile([P, 25], F32, tag="t1")
            t2_sb = pp.tile([16, 384], F32, tag="t2")
            pr_sb = pp.tile([P, 18], F32, tag="pr")
            nc.gpsimd.dma_start(out=t1_sb[:], in_=t1_d[:])
            nc.gpsimd.dma_start(out=t2_sb[:], in_=t2_d[:])
            nc.gpsimd.dma_start(out=pr_sb[:], in_=params_d[:])
            gx_sb = t1_sb[:, 0:16]
            gc_sb = t1_sb[:, 16:24]
            ones_col_f32 = t1_sb[:, 24:25]
            gxt_sb = t2_sb[:, 0:128]
            gct_sb = t2_sb[0:8, 128:256]
            ones_row_f32 = t2_sb[0:1, 256:384]
            gnxw = pr_sb[:, 0:2]
            gnxb = pr_sb[:, 2:4]
            qb_sb = pr_sb[:, 4:6]
            kb_sb = pr_sb[:, 6:8]
            gncw = pr_sb[:, 8:12]
            gncb = pr_sb[:, 12:16]
            vb_sb = pr_sb[:, 16:18]
            ones_row_r = pp.tile([1, P], F32R, tag="ones_row_r")
            ones8 = pp.tile([P, 2, 32], FP8, tag="ones8")
            c4096 = pp.tile([1, 1], BF16, tag="c4096")
            ones512b = pp.tile([1, 512], BF16, tag="ones512b")
            nc.vector.memset(c4096[:], float(N))
            nc.vector.memset(ones512b[:], 1.0)
            eps_sb = pp.tile([P, 1], F32, tag="eps")
            with nc.allow_low_precision("f32r ones"):
                nc.vector.tensor_copy(ones_row_r[:], ones_row_f32[:])
            nc.vector.memset(ones8[:], 1.0)
            nc.vector.memset(eps_sb[:], EPS)

            NSUB = 8  # bn_stats free-dim limit is 512

            q_all = pp.tile([P, CKX, N], FP8, tag="q_all")
            k_all = pp.tile([P, CKX, N], FP8, tag="k_all")
            vt_all = pp.tile([P, MI, C], FP8, tag="vt_all")

            qb_f = pp.tile([P, CKX], F32, tag="qb_f")
            kb_f = pp.tile([P, CKX], F32, tag="kb_f")
            vsum = pp.tile([P, CKX], F32, tag="vsum")
            vbt = pp.tile([P, CKX], F32, tag="vbt")

            # x_r and wq_r persist: the q projection for nj+1 is pipelined
            # into the attention loop of nj.
            x_r = pp.tile([P, CKX, N], BF16, tag="x_r")
            wq_r = pp.tile([P, CKX, C], BF16, tag="wq_r")

            pj_stack = ExitStack()
            pj = pj_stack.enter_context(tc.tile_pool(name="projsb", bufs=1))
            head_stack = ExitStack()
            scp = head_stack.enter_context(tc.tile_pool(name="scratch", bufs=2))
            psp = head_stack.enter_context(
                tc.tile_pool(name="psum_small", bufs=1, space="PSUM"))
            phd = head_stack.enter_context(
                tc.tile_pool(name="psum_head", bufs=4, space="PSUM"))
            pwv = head_stack.enter_context(
                tc.tile_pool(name="psum_wave", bufs=3, space="PSUM"))
            if True:
                c_r = pj.tile([P, CKC, N], F32R, tag="c_r")
                wk_r = pj.tile([P, CKC, C], F32R, tag="wk_r")
                wv_r = pj.tile([P, CKC, C], F32R, tag="wv_r")
                alpha_x = pj.tile([P, CKX], F32, tag="alpha_x")
                beta_x = pj.tile([P, CKX], F32, tag="beta_x")
                alpha_c = pj.tile([P, CKC], F32, tag="alpha_c")
                beta_c = pj.tile([P, CKC], F32, tag="beta_c")
                rawcs = pj.tile([P, CKC], F32, tag="rawcs")  # 4096*raw mean
                rawcs2 = pj.tile([P, CKC], F32, tag="rawcs2")  # alpha-scaled

                # raw fp32 weights: dead after the bias folds in the head
                wq_f = pj.tile([P, CKX, C], F32, tag="wq_f")
                wk_f = pj.tile([P, CKC, C], F32, tag="wk_f")
                wv_f = pj.tile([P, CKC, C], F32, tag="wv_f")
                with (tc.tile_pool(name="raw", bufs=2) as rawp,
                      tc.tile_pool(name="rawx", bufs=2) as rawxp):

                    def affine_tail(me, k, g_sb, gt_sb, gpc, d_per_g,
                                    w_sb, b_sb, alpha, beta):
                        gs = psp.tile([gpc, 2], F32, tag="aff")
                        nc.tensor.matmul(gs[:], g_sb[:, :], me[:],
                                         start=True, stop=True)
                        mv = smp.tile([gpc, 2], F32, tag="mv")
                        nc.vector.tensor_scalar_mul(
                            out=mv[:], in0=gs[:], scalar1=1.0 / d_per_g
                        )
                        msq = smp.tile([gpc, 1], F32, tag="msq")
                        nc.vector.tensor_mul(msq[:], mv[:, 0:1], mv[:, 0:1])
                        var = smp.tile([gpc, 1], F32, tag="var")
                        nc.vector.tensor_sub(var[:], mv[:, 1:2], msq[:])
                        sd = smp.tile([gpc, 1], F32, tag="sd")
                        nc.scalar.activation(
                            out=sd[:], in_=var[:], func=AF.Sqrt,
                            bias=eps_sb[:gpc], scale=1.0,
                        )
                        mv2 = smp.tile([gpc, 2], F32, tag="mv2")
                        nc.vector.tensor_copy(mv2[:, 0:1], mv[:, 0:1])
                        nc.vector.reciprocal(mv2[:, 1:2], sd[:])
                        murs = psp.tile([P, 2], F32, tag="aff")
                        nc.tensor.matmul(
                            murs[:], gt_sb[:, :], mv2[:], start=True, stop=True
                        )
                        nc.vector.tensor_mul(
                            alpha[:, k : k + 1], murs[:, 1:2], w_sb[:, k : k + 1]
                        )
                        t1v = smp.tile([P, 1], F32, tag="t1v")
                        nc.vector.tensor_mul(t1v[:], murs[:, 0:1],
                                             alpha[:, k : k + 1])
                        nc.vector.tensor_sub(
                            beta[:, k : k + 1], b_sb[:, k : k + 1], t1v[:]
                        )

                    def process_chunk(dram, k, r_dst, g_sb, gt_sb, gpc,
                                      d_per_g, w_sb, b_sb, alpha, beta,
                                      mean_dst, queue_eng, round_act=False,
                                      defer_affine=False, use_rawx=False,
                                      defer_round=False):
                        # cond path: bn stats on DVE, f32r rounding on GPSIMD
                        H = N // 2
                        rp = rawxp if use_rawx else rawp
                        raw_a = rp.tile([P, H], F32, tag="rawa")
                        raw_b = rp.tile([P, H], F32, tag="rawb")
                        queue_eng.dma_start(
                            out=raw_a[:], in_=dram[k * P : (k + 1) * P, 0:H]
                        )
                        queue_eng.dma_start(
                            out=raw_b[:], in_=dram[k * P : (k + 1) * P, H:N]
                        )
                        bn = scp.tile([P, NSUB, 6], F32, tag="bn")
                        for s in range(NSUB):
                            src = raw_a if s < NSUB // 2 else raw_b
                            off = s % (NSUB // 2)
                            nc.vector.bn_stats(
                                out=bn[:, s, :],
                                in_=src[:, off * 512 : (off + 1) * 512],
                            )
                        if defer_round:
                            deferred_rounds.append((r_dst, k, raw_a, raw_b))
                        with nc.allow_low_precision("f32r inputs"):
                            if defer_round:
                                pass
                            elif round_act:
                                nc.scalar.activation(
                                    out=r_dst[:, k, 0:H], in_=raw_a[:],
                                    func=AF.Copy, scale=1.0)
                                nc.scalar.activation(
                                    out=r_dst[:, k, H:N], in_=raw_b[:],
                                    func=AF.Copy, scale=1.0)
                            else:
                                nc.gpsimd.tensor_copy(r_dst[:, k, 0:H],
                                                      raw_a[:])
                                nc.gpsimd.tensor_copy(r_dst[:, k, H:N],
                                                      raw_b[:])
                        mvp = scp.tile([P, 2], F32, tag="mvp")
                        nc.vector.bn_aggr(out=mvp[:], in_=bn[:])
                        if mean_dst is not None:
                            nc.vector.tensor_scalar(
                                out=mean_dst[:, k : k + 1], in0=mvp[:, 0:1],
                                scalar1=float(N), scalar2=None, op0=OP.mult,
                            )
                        # me = [mean, E[x^2]] per partition
                        me = scp.tile([P, 2], F32, tag="me",
                                      name=f"me_{dram.name}_{k}")
                        nc.vector.tensor_copy(me[:, 0:1], mvp[:, 0:1])
                        nc.vector.scalar_tensor_tensor(
                            out=me[:, 1:2], in0=mvp[:, 0:1], scalar=mvp[:, 0:1],
                            in1=mvp[:, 1:2], op0=OP.mult, op1=OP.add,
                        )
                        if defer_affine:
                            return me
                        affine_tail(me, k, g_sb, gt_sb, gpc, d_per_g,
                                    w_sb, b_sb, alpha, beta)

                    # cond chunks: DMAs alternate between the SP and ACT
                    # hwdge rings so two chunks stream in parallel.  Wave
                    # k-proj tiles accumulate per chunk to fill PE.
                    NWAVE = 3
                    wave_ps = [pwv.tile([P, 512], F32, tag="wave",
                                        name=f"wave_ps{w}")
                               for w in range(NWAVE)]
                    for k in range(CKC):
                        process_chunk(cond_d, k, c_r, gc_sb, gct_sb, P // DC,
                                      DC, gncw, gncb, alpha_c, beta_c, rawcs,
                                      nc.sync, round_act=(k == CKC - 1))
                        nc.sync.dma_start(
                            out=wk_f[:, k, :], in_=wk_d[k * P : (k + 1) * P, :])
                        with tc.high_priority():
                            with nc.allow_low_precision("f32r weights"):
                                nc.vector.tensor_scalar_mul(
                                    out=wk_r[:, k, :], in0=wk_f[:, k, :],
                                    scalar1=alpha_c[:, k : k + 1],
                                )
                        for w in range(NWAVE):
                            nc.tensor.matmul(
                                wave_ps[w][:],
                                wk_r[:, k, 0:P],
                                c_r[:, k, w * 512 : (w + 1) * 512],
                                start=(k == 0), stop=(k == CKC - 1),
                            )

                    for k in range(CKC):
                        nc.sync.dma_start(
                            out=wv_f[:, k, :], in_=wv_d[k * P : (k + 1) * P, :])
                    for k in range(CKX):
                        nc.sync.dma_start(
                            out=wq_f[:, k, :], in_=wq_d[k * P : (k + 1) * P, :])
                    # x chunks: DVE stats after cond, ACT rounding;
                    # the PE-dependent affine tail is deferred until after
                    # the k projection so it never blocks the PE queue
                    x_mes = []
                    deferred_rounds = []
                    for k in range(CKX):
                        x_mes.append(process_chunk(
                            x_d, k, x_r, gx_sb, gxt_sb,
                            P // DX, DX, gnxw, gnxb,
                            alpha_x, beta_x, None, nc.sync,
                            round_act=True, defer_affine=True,
                            use_rawx=True, defer_round=True))

                    with tc.high_priority():
                        with nc.allow_low_precision("f32r weights"):
                            for k in range(CKC):
                                nc.vector.tensor_scalar_mul(
                                    out=wv_r[:, k, :], in0=wv_f[:, k, :],
                                    scalar1=alpha_c[:, k : k + 1],
                                )

                    # k_b' fold and v-constants (cond-only)
                    for co in range(CKX):
                        bk = psp.tile([P, 1], F32, tag="aff")
                        for ci in range(CKC):
                            nc.tensor.matmul(
                                bk[:],
                                wk_f[:, ci, co * P : (co + 1) * P],
                                beta_c[:, ci : ci + 1],
                                start=(ci == 0), stop=(ci == CKC - 1),
                            )
                        nc.vector.tensor_add(
                            kb_f[:, co : co + 1], bk[:], kb_sb[:, co : co + 1]
                        )
                    # vb_tot[c] = v_b + Wv' beta_c ; Vsum[c] = Wv_r' rawcs
                    for co in range(CKX):
                        bv = psp.tile([P, 1], F32, tag="aff")
                        for ci in range(CKC):
                            nc.tensor.matmul(
                                bv[:],
                                wv_f[:, ci, co * P : (co + 1) * P],
                                beta_c[:, ci : ci + 1],
                                start=(ci == 0), stop=(ci == CKC - 1),
                            )
                        nc.vector.tensor_add(
                            vbt[:, co : co + 1], bv[:], vb_sb[:, co : co + 1]
                        )
                        if co == 0:
                            nc.vector.tensor_mul(rawcs2[:], rawcs[:],
                                                 alpha_c[:])
                        sv = psp.tile([P, 1], F32, tag="aff")
                        for ci in range(CKC):
                            nc.tensor.matmul(
                                sv[:],
                                wv_f[:, ci, co * P : (co + 1) * P],
                                rawcs2[:, ci : ci + 1],
                                start=(ci == 0), stop=(ci == CKC - 1),
                            )
                        nc.vector.tensor_copy(vsum[:, co : co + 1], sv[:])

                    # evict the wave tiles, then the remaining k-proj
                    for w in range(NWAVE):
                        nc.scalar.activation(
                            out=k_all[:, 0, w * 512 : (w + 1) * 512],
                            in_=wave_ps[w][:], func=AF.Identity,
                            bias=kb_f[:, 0:1], scale=1.0,
                        )
                    def krest_emit(co, nj, idx):
                        ps = phd.tile([P, 512], F32, tag="proj")
                        for ci in range(CKC):
                            nc.tensor.matmul(
                                ps[:],
                                wk_r[:, ci, co * P : (co + 1) * P],
                                c_r[:, ci, nj * 512 : (nj + 1) * 512],
                                start=(ci == 0), stop=(ci == CKC - 1),
                            )
                        if idx % 2 == 0:
                            nc.scalar.activation(
                                out=k_all[:, co, nj * 512 : (nj + 1) * 512],
                                in_=ps[:], func=AF.Identity,
                                bias=kb_f[:, co : co + 1], scale=1.0,
                            )
                        else:
                            nc.vector.tensor_scalar(
                                out=k_all[:, co, nj * 512 : (nj + 1) * 512],
                                in0=ps[:], scalar1=kb_f[:, co : co + 1],
                                scalar2=None, op0=OP.add,
                            )

                    def vproj_emit(pair, idx):
                        # two m-chunks share one psum tile and one eviction
                        ps = phd.tile([P, 2, C], F32, tag="proj")
                        for h in range(2):
                            mi = 2 * pair + h
                            for ci in range(CKC):
                                nc.tensor.matmul(
                                    ps[:, h, :],
                                    c_r[:, ci, mi * P : (mi + 1) * P],
                                    wv_r[:, ci, :],
                                    start=(ci == 0), stop=(ci == CKC - 1),
                                )
                        if idx % 2 == 0:
                            nc.vector.tensor_scalar(
                                out=vt_all[:, 2 * pair : 2 * pair + 2, :],
                                in0=ps[:],
                                scalar1=1.0, scalar2=None, op0=OP.mult,
                            )
                        else:
                            nc.scalar.activation(
                                out=vt_all[:, 2 * pair : 2 * pair + 2, :],
                                in_=ps[:], func=AF.Copy, scale=1.0,
                            )

                    krest = sorted(
                        ((co, nj) for co in range(CKX) for nj in range(NJ)
                         if not (co == 0 and nj < NWAVE)),
                        key=lambda t: (t[1] >= NJ // 2, t[1], t[0]))
                    idx = 0
                    vq = list(range(MI // 4))
                    for i, (co, nj) in enumerate(krest):
                        krest_emit(co, nj, idx); idx += 1
                        if i % 2 == 1 and vq:
                            vproj_emit(vq.pop(0), idx); idx += 1

                    # deferred x rounds: emitted after the k/v eviction
                    # chains so they never block the ACT queue while x still
                    # streams in
                    H2 = N // 2
                    for r_dst, kx, raw_a, raw_b in deferred_rounds:
                        with nc.allow_low_precision("f32r inputs"):
                            nc.scalar.activation(
                                out=r_dst[:, kx, 0:H2], in_=raw_a[:],
                                func=AF.Copy, scale=1.0)
                            nc.scalar.activation(
                                out=r_dst[:, kx, H2:N], in_=raw_b[:],
                                func=AF.Copy, scale=1.0)

                    # deferred x affine tails + q-weight prep
                    for k in range(CKX):
                        affine_tail(x_mes[k], k, gx_sb, gxt_sb, P // DX, DX,
                                    gnxw, gnxb, alpha_x, beta_x)
                    with nc.allow_low_precision("f32r weights"):
                        for k in range(CKX):
                            nc.vector.tensor_scalar_mul(
                                out=wq_r[:, k, :], in0=wq_f[:, k, :],
                                scalar1=alpha_x[:, k : k + 1],
                            )
                    for co in range(CKX):
                        bq = psp.tile([P, 1], F32, tag="aff")
                        for ci in range(CKX):
                            nc.tensor.matmul(
                                bq[:],
                                wq_f[:, ci, co * P : (co + 1) * P],
                                beta_x[:, ci : ci + 1],
                                start=(ci == 0), stop=(ci == CKX - 1),
                            )
                        nc.vector.tensor_add(
                            qb_f[:, co : co + 1], bq[:], qb_sb[:, co : co + 1]
                        )

                while vq:
                    vproj_emit(vq.pop(0), idx)
                    idx += 1
                # q projection for nj=0 only (the rest pipeline into the
                # attention loop); ACT eviction (ACT idle pre-attention)
                for co in range(CKX):
                    ps = phd.tile([P, 512], F32, tag="proj")
                    for ci in range(CKX):
                        nc.tensor.matmul(
                            ps[:],
                            wq_r[:, ci, co * P : (co + 1) * P],
                            x_r[:, ci, 0:512],
                            start=(ci == 0), stop=(ci == CKX - 1),
                        )
                    nc.scalar.activation(
                        out=q_all[:, co, 0:512],
                        in_=ps[:], func=AF.Identity,
                        bias=qb_f[:, co : co + 1], scale=1.0,
                    )

            head_stack.close()

            # ---- attention ----
            # Steady state is paced by ACT (one 1024-wide exp per m-pair).
            # The nj epilogue is deferred into the first steps of nj+1 so
            # the PE/DVE queues never stall behind the recip chain, and the
            # q projection for nj+1 runs mid-loop on PE/DVE slack.
            with (
                tc.tile_pool(name="psum_s", bufs=2, space="PSUM") as pss,
                tc.tile_pool(name="psum_o", bufs=1, space="PSUM") as pso,
                tc.tile_pool(name="psum_t", bufs=1, space="PSUM") as pst,
                tc.tile_pool(name="exps", bufs=4) as pex,
                tc.tile_pool(name="deltas", bufs=4) as pdl,
                tc.tile_pool(name="outs", bufs=1) as pout,
            ):
                SKEW = 2
                prev = None  # (nj, o0, o1, den_ps)

                def tail_a(state):
                    nj0, o0, o1, den_ps = state
                    recip = smp.tile([1, 512], F32R, tag="recip",
                                     name=f"recip{nj0}")
                    with nc.allow_low_precision("f32r reciprocal"):
                        nc.vector.reciprocal(recip[:], den_ps[0:1, :])
                    return recip

                def tail_b(state, recip):
                    nj0 = state[0]
                    bc_ps = pst.tile([P, 512], F32, tag="aux",
                                     name=f"bc{nj0}")
                    nc.tensor.matmul(
                        bc_ps[:], ones_row_r[:], recip[:],
                        start=True, stop=True,
                    )
                    bc_sb = pout.tile([P, 512], F32, tag="bc_sb",
                                      name=f"bcsb{nj0}")
                    nc.vector.tensor_copy(bc_sb[:], bc_ps[:])
                    return bc_sb

                def tail_c(state, bc_sb):
                    nj0, o0, o1, den_ps = state
                    for co in range(CKX):
                        o_sb = pout.tile([P, 512], F32, tag=f"osb{co}",
                                         name=f"osb{nj0}_{co}")
                        nc.vector.scalar_tensor_tensor(
                            out=o_sb[:], in0=(o0 if co == 0 else o1)[:],
                            scalar=vsum[:, co : co + 1], in1=bc_sb[:],
                            op0=OP.add, op1=OP.mult,
                        )
                        f_sb = pout.tile([P, 512], F32, tag=f"fsb{co}",
                                         name=f"fsb{nj0}_{co}")
                        nc.gpsimd.tensor_scalar(
                            out=f_sb[:], in0=o_sb[:],
                            scalar1=vbt[:, co : co + 1], scalar2=None,
                            op0=OP.add,
                        )
                        nc.sync.dma_start(
                            out=out_d[co * P : (co + 1) * P,
                                      nj0 * 512 : (nj0 + 1) * 512],
                            in_=f_sb[:],
                        )

                def qproj_emit(nj2, co):
                    ps = pst.tile([P, 512], F32, tag="aux",
                                  name=f"qproj{nj2}_{co}")
                    for ci in range(CKX):
                        nc.tensor.matmul(
                            ps[:],
                            wq_r[:, ci, co * P : (co + 1) * P],
                            x_r[:, ci, nj2 * 512 : (nj2 + 1) * 512],
                            start=(ci == 0), stop=(ci == CKX - 1),
                        )
                    nc.vector.tensor_scalar(
                        out=q_all[:, co, nj2 * 512 : (nj2 + 1) * 512],
                        in0=ps[:], scalar1=qb_f[:, co : co + 1],
                        scalar2=None, op0=OP.add,
                    )

                # One flat software pipeline over (nj, pair): the S/exp/
                # delta stream runs SKEW pairs ahead of the O/den stream,
                # crossing nj boundaries without draining.
                TOT = NJ * NP
                d_tiles = [None] * TOT
                o_tiles = {}
                recip = bc_sb = None
                for gs_ in range(TOT + SKEW):
                    if gs_ < TOT:
                        nj, pi = divmod(gs_, NP)
                        ncol = slice(nj * 512, (nj + 1) * 512)
                        s_pair = pss.tile([P, 2, 512], F32, tag="s")
                        for h in range(2):
                            mi = 2 * pi + h
                            nc.tensor.matmul(
                                s_pair[:, h, :],
                                k_all[:, :, mi * P : (mi + 1) * P],
                                q_all[:, :, ncol],
                                start=True, stop=True,
                                perf_mode=PM.DoubleRow,
                            )
                        e_pair = pex.tile([P, 2, 512], BF16, tag="e")
                        nc.scalar.activation(
                            out=e_pair[:], in_=s_pair[:], func=AF.Exp,
                            scale=SOFTMAX_SCALE,
                        )
                        d_pair = pdl.tile([P, 2, 512], FP8, tag="d")
                        nc.vector.tensor_scalar(
                            out=d_pair[:], in0=e_pair[:],
                            scalar1=1.0, scalar2=None, op0=OP.subtract,
                        )
                        d_tiles[gs_] = d_pair
                        # remaining v projection + q projection for
                        # nj+1 on PE/DVE slack
                        if nj == 0:
                            mi2 = 16 + pi
                            ps2 = pst.tile([P, C], F32, tag="aux",
                                           name=f"vproj{mi2}")
                            for ci in range(CKC):
                                nc.tensor.matmul(
                                    ps2[:],
                                    c_r[:, ci, mi2 * P : (mi2 + 1) * P],
                                    wv_r[:, ci, :],
                                    start=(ci == 0), stop=(ci == CKC - 1),
                                )
                            nc.vector.tensor_scalar(
                                out=vt_all[:, mi2, :], in0=ps2[:],
                                scalar1=1.0, scalar2=None, op0=OP.mult,
                            )
                        if nj < NJ - 1 and pi in (10, 12):
                            qproj_emit(nj + 1, 0 if pi == 10 else 1)
                    if gs_ >= SKEW:
                        go = gs_ - SKEW
                        nj, pi = divmod(go, NP)
                        if pi == 0:
                            # deferred epilogue of the previous nj
                            if prev is not None:
                                recip = tail_a(prev)
                                bc_sb = tail_b(prev, recip)
                                tail_c(prev, bc_sb)
                            o0 = pso.tile([P, 512], F32, tag="o0")
                            o1 = pso.tile([P, 512], F32, tag="o1")
                            den_ps = pst.tile([32, 512], F32, tag="den",
                                              name=f"den{nj}")
                            o_tiles[nj] = (nj, o0, o1, den_ps)
                        nj_, o0, o1, den_ps = o_tiles[nj]
                        d_pair = d_tiles[go]
                        st = pi == 0
                        sp = pi == NP - 1
                        nc.tensor.matmul(
                            o0[:], vt_all[:, 2 * pi : 2 * pi + 2, 0:P],
                            d_pair[:], start=st, stop=sp,
                            perf_mode=PM.DoubleRow,
                        )
                        nc.tensor.matmul(
                            o1[:], vt_all[:, 2 * pi : 2 * pi + 2, P:C],
                            d_pair[:], start=st, stop=sp,
                            perf_mode=PM.DoubleRow,
                        )
                        nc.tensor.matmul(
                            den_ps[:], ones8[:],
                            d_pair[:], start=st, stop=False,
                            perf_mode=PM.DoubleRow,
                        )
                        d_tiles[go] = None
                        if sp:
                            # fold the +4096 softmax-denominator constant
                            # into the accumulation group on PE
                            nc.tensor.matmul(
                                den_ps[0:1, :], c4096[:], ones512b[:],
                                start=False, stop=True,
                                skip_group_check=True,
                            )
                            prev = o_tiles.pop(nj)
                # final epilogue
                # final epilogue, split into column halves so the
                # serial den->recip->bc->scale chain pipelines
                nj0_, o0, o1, den_ps = prev
                for hh in range(2):
                    hs = slice(hh * 256, (hh + 1) * 256)
                    recip = smp.tile([1, 256], F32R, tag="recip",
                                     name=f"recipf{hh}")
                    with nc.allow_low_precision("f32r reciprocal"):
                        nc.vector.reciprocal(recip[:], den_ps[0:1, hs])
                    bc_ps = pst.tile([P, 256], F32, tag="aux",
                                     name=f"bcf{hh}")
                    nc.tensor.matmul(
                        bc_ps[:], ones_row_r[:], recip[:],
                        start=True, stop=True,
                    )
                    bc_sb = pout.tile([P, 256], F32, tag="bc_sb",
                                      name=f"bcsbf{hh}")
                    nc.vector.tensor_copy(bc_sb[:], bc_ps[:])
                    for co in range(CKX):
                        o_sb = pout.tile([P, 256], F32, tag=f"osb{co}",
                                         name=f"osbf{hh}_{co}")
                        nc.vector.scalar_tensor_tensor(
                            out=o_sb[:], in0=(o0 if co == 0 else o1)[:, hs],
                            scalar=vsum[:, co : co + 1], in1=bc_sb[:],
                            op0=OP.add, op1=OP.mult,
                        )
                        f_sb = pout.tile([P, 256], F32, tag=f"fsb{co}",
                                         name=f"fsbf{hh}_{co}")
                        nc.vector.tensor_scalar(
                            out=f_sb[:], in0=o_sb[:],
                            scalar1=vbt[:, co : co + 1], scalar2=None,
                            op0=OP.add,
                        )
                        nc.sync.dma_start(
                            out=out_d[co * P : (co + 1) * P,
                                      nj0_ * 512 + hh * 256 :
                                      nj0_ * 512 + (hh + 1) * 256],
                            in_=f_sb[:],
                        )
            pj_stack.close()

    nc.finalize()
    if fixup:
        _split_multiwait_instructions(nc)
    return nc


def pack_params(gn_x_w, gn_x_b, q_b, k_b, gn_c_w, gn_c_b, v_b):
    pr = np.zeros((P, 18), np.float32)
    pr[:, 0:2] = np.asarray(gn_x_w, np.float32).reshape(2, P).T
    pr[:, 2:4] = np.asarray(gn_x_b, np.float32).reshape(2, P).T
    pr[:, 4:6] = np.asarray(q_b, np.float32).reshape(2, P).T
    pr[:, 6:8] = np.asarray(k_b, np.float32).reshape(2, P).T
    pr[:, 8:12] = np.asarray(gn_c_w, np.float32).reshape(4, P).T
    pr[:, 12:16] = np.asarray(gn_c_b, np.float32).reshape(4, P).T
    pr[:, 16:18] = np.asarray(v_b, np.float32).reshape(2, P).T
    return pr


def _get_nc():
    if "nc" not in _CACHE:
        _CACHE["nc"] = build_module()
    return _CACHE["nc"]


def kernel(x, condA, gn_x_w, gn_x_b, gn_c_w, gn_c_b,
           q_w, q_b, k_w, k_b, v_w, v_b):
    x = np.asarray(x, np.float32)
    condA = np.asarray(condA, np.float32)
    wq_t = np.ascontiguousarray(np.asarray(q_w, np.float32).T)
    wk_t = np.ascontiguousarray(np.asarray(k_w, np.float32).T)
    wv_t = np.ascontiguousarray(np.asarray(v_w, np.float32).T)
    shared = {
        "wq_t": wq_t,
        "wk_t": wk_t,
        "wv_t": wv_t,
        "params": pack_params(gn_x_w, gn_x_b, q_b, k_b, gn_c_w, gn_c_b, v_b),
    }
    in_maps = []
    for b in range(B):
        m = dict(shared)
        m["x"] = np.ascontiguousarray(x[b].reshape(C, N))
        m["cond"] = np.ascontiguousarray(condA[b].reshape(E, N))
        in_maps.append(m)

    nc = _get_nc()
    res = run_bass_kernel_spmd(nc, in_maps, core_ids=list(range(B)))
    out = np.stack([res.results[b]["out"] for b in range(B)], axis=0)
    return out.reshape(B, C, 64, 64)


if __name__ == "__main__":
    rng = np.random.default_rng(0)
    ins = {
        "x": rng.standard_normal((B, C, 64, 64), dtype=np.float32),
        "condA": rng.standard_normal((B, E, 64, 64), dtype=np.float32),
        "gn_x_w": np.ones(C, np.float32),
        "gn_x_b": np.zeros(C, np.float32),
        "gn_c_w": np.ones(E, np.float32),
        "gn_c_b": np.zeros(E, np.float32),
        "q_w": (rng.standard_normal((C, C)) * 0.02).astype(np.float32),
        "q_b": np.zeros(C, np.float32),
        "k_w": (rng.standard_normal((C, E)) * 0.02).astype(np.float32),
        "k_b": np.zeros(C, np.float32),
        "v_w": (rng.standard_normal((C, E)) * 0.02).astype(np.float32),
        "v_b": np.zeros(C, np.float32),
    }
    o = kernel(**ins)
    print("out", o.shape, o.dtype, float(np.abs(o).max()))
